# revision 1
# baseline (speedup 1.0000x reference)
"""GIN message-passing GNN (6 layers) on 8 Trainium2 NeuronCores.

Sharding: edges partitioned by dst into 8 node-shards (6250 dst rows each).
Each core: dma_gather(h[src]) for its edges -> one-hot matmul segment-sum per
128-dst block (+ self term via PE transpose) -> MLP in feature-major layout ->
transposed write of its h shard -> AllGather replicates h for the next layer.

Self-contained: hardcodes all shapes; builds/compiles the Bass program on
first call, specialized to the runtime edge structure.
"""
import math
import sys

import numpy as np

sys.path.insert(0, "/opt/trn_rl_repo")
sys.path.insert(0, "/root/problem")

N, E = 50000, 800000
IN, H, OUT = 128, 128, 64
N_MID = 4
NCORES = 8
SHARD = N // NCORES          # 6250
NBLK = math.ceil(SHARD / 128)  # 49 blocks per core (last = 106 rows)
LAST_ROWS = SHARD - 128 * (NBLK - 1)
SPLIT = 32768                # int16 gather index limit; h split at this row
MAX_G = 8                    # max ktiles (1024 idxs) per dma_gather call


def _wrap_idxs_flat(idx_vals: np.ndarray) -> np.ndarray:
    """[n] -> [128, n//16] int16 (16-partition wrap, replicated 8x)."""
    n = len(idx_vals)
    assert n % 16 == 0
    buf = np.zeros((16, n // 16), dtype=np.int16)
    ar = np.arange(n)
    buf[ar % 16, ar // 16] = idx_vals.astype(np.int16)
    return np.tile(buf, (8, 1))


def _prep_edges(edge_index: np.ndarray):
    """Partition/sort/pad edges. Returns per-core idx tables, dstrel tables,
    and the uniform per-block ktile counts (K_LO, K_HI)."""
    # GIN agg = h_i + sum_{j->i} h_j: fold the self term in as edges (i->i),
    # which also sidesteps per-core self-row addressing in the SPMD program.
    self_edges = np.arange(N, dtype=np.int64)
    src = np.concatenate([edge_index[0].astype(np.int64), self_edges])
    dst = np.concatenate([edge_index[1].astype(np.int64), self_edges])
    core_of = dst // SHARD
    per_core = []
    K_LO = K_HI = 0
    for k in range(NCORES):
        m = core_of == k
        s, d = src[m], dst[m] - k * SHARD
        blk = d // 128
        lo_lists, hi_lists = [], []
        for b in range(NBLK):
            mb_ = blk == b
            sb, db = s[mb_], d[mb_] - 128 * b
            lo = sb < SPLIT
            lo_lists.append((sb[lo], db[lo]))
            hi_lists.append((sb[~lo] - SPLIT, db[~lo]))
            K_LO = max(K_LO, math.ceil(max(len(sb[lo]), 1) / 128))
            K_HI = max(K_HI, math.ceil(max(len(sb[~lo]), 1) / 128))
        per_core.append((lo_lists, hi_lists))

    KT = K_LO + K_HI  # ktiles per block
    idx_tables, dstrel_tables = [], []
    for k in range(NCORES):
        lo_lists, hi_lists = per_core[k]
        idx_flat = np.zeros(NBLK * KT * 128, dtype=np.int64)
        rel_flat = np.full(NBLK * KT * 128, -1.0, dtype=np.float32)
        for b in range(NBLK):
            base = b * KT * 128
            for (sb, db), off, kcnt in (
                (lo_lists[b], 0, K_LO),
                (hi_lists[b], K_LO * 128, K_HI),
            ):
                n = len(sb)
                assert n <= kcnt * 128
                idx_flat[base + off: base + off + n] = sb
                rel_flat[base + off: base + off + n] = db.astype(np.float32)
        idx_tables.append(_wrap_idxs_flat(idx_flat))
        # dstrel layout [128, NBLK*KT]: slot j -> partition j%128, col j//128
        dstrel_tables.append(
            rel_flat.reshape(NBLK * KT, 128).T.copy()
        )
    return idx_tables, dstrel_tables, K_LO, K_HI


_CACHE = {}


def _build(K_LO: int, K_HI: int):
    from concourse import bacc, mybir, library_config
    from concourse.tile import TileContext

    KT = K_LO + K_HI
    nc = bacc.Bacc("TRN2", target_bir_lowering=False, debug=False,
                   num_devices=NCORES)
    f32 = mybir.dt.float32

    x_in = nc.declare_dram_parameter("x", [N, IN], f32, isOutput=False)
    idxs_in = nc.declare_dram_parameter("idxs", [128, NBLK * KT * 8], mybir.dt.int16, isOutput=False)
    dstrel_in = nc.declare_dram_parameter("dstrel", [128, NBLK * KT], f32, isOutput=False)
    iota_in = nc.declare_dram_parameter("iota", [128, 128], f32, isOutput=False)
    ident_in = nc.declare_dram_parameter("ident", [128, 128], f32, isOutput=False)
    # weights, feature-major conventions: wa[l] used as lhsT [in,H]
    wa_in = nc.declare_dram_parameter("wa", [5, 128, 128], f32, isOutput=False)
    wb_in = nc.declare_dram_parameter("wb", [5, 128, 128], f32, isOutput=False)
    ba_in = nc.declare_dram_parameter("ba", [5, 128], f32, isOutput=False)
    bb_in = nc.declare_dram_parameter("bb", [5, 128], f32, isOutput=False)
    wl_in = nc.declare_dram_parameter("wl", [128, OUT], f32, isOutput=False)
    bl_in = nc.declare_dram_parameter("bl", [OUT], f32, isOutput=False)
    out_ext = nc.declare_dram_parameter("out", [SHARD, OUT], f32, isOutput=True)

    ag_in = nc.dram_tensor("ag_in", [SHARD, H], f32)
    hbufs = [nc.dram_tensor(f"h{i}", [N, H], f32, addr_space="Shared") for i in range(2)]

    with TileContext(nc) as tc:
        with tc.tile_pool(name="cst", bufs=1) as cst, \
             tc.tile_pool(name="gat", bufs=6) as gat, \
             tc.tile_pool(name="ahot", bufs=4) as ahot, \
             tc.tile_pool(name="work", bufs=4) as work, \
             tc.tile_pool(name="psum", bufs=3, space="PSUM") as ps, \
             tc.tile_pool(name="psmlp", bufs=1, space="PSUM") as psm:
            nc.gpsimd.load_library(library_config.mlp)
            idx_t = cst.tile([128, NBLK * KT * 8], mybir.dt.int16)
            nc.sync.dma_start(out=idx_t[:], in_=idxs_in[:, :])
            dstrel_t = cst.tile([128, NBLK * KT], f32)
            nc.sync.dma_start(out=dstrel_t[:], in_=dstrel_in[:, :])
            iota_t = cst.tile([128, 128], f32)
            nc.sync.dma_start(out=iota_t[:], in_=iota_in[:, :])
            ident_t = cst.tile([128, 128], f32)
            nc.sync.dma_start(out=ident_t[:], in_=ident_in[:, :])
            wa_t = cst.tile([128, 5, 128], f32)
            nc.sync.dma_start(out=wa_t[:], in_=wa_in[:, :, :].rearrange("l p d -> p l d"))
            wb_t = cst.tile([128, 5, 128], f32)
            nc.sync.dma_start(out=wb_t[:], in_=wb_in[:, :, :].rearrange("l p d -> p l d"))
            ba_t = cst.tile([128, 5], f32)
            nc.sync.dma_start(out=ba_t[:], in_=ba_in[:, :].rearrange("l p -> p l"))
            bb_t = cst.tile([128, 5], f32)
            nc.sync.dma_start(out=bb_t[:], in_=bb_in[:, :].rearrange("l p -> p l"))
            wl_t = cst.tile([128, OUT], f32)
            nc.sync.dma_start(out=wl_t[:], in_=wl_in[:, :])
            bl_t = cst.tile([OUT, 1], f32)
            nc.sync.dma_start(out=bl_t[:], in_=bl_in[:, None])

            for layer in range(6):
                if layer == 0:
                    h_src = x_in
                else:
                    h_src = hbufs[(layer - 1) % 2]
                h_dst = hbufs[layer % 2]
                final = layer == 5
                wcols = OUT if final else H

                sc = nc.named_scope(f"L{layer}")
                sc.__enter__()
                for b in range(NBLK):
                    rows = LAST_ROWS if b == NBLK - 1 else 128
                    kt_base = b * KT
                    # --- gathers (split into <=MAX_G-ktile calls) ---
                    g_t = gat.tile([128, KT, H], f32, tag="gt")
                    for part_off, part_kt, src_off in ((0, K_LO, 0), (K_LO, K_HI, SPLIT)):
                        done = 0
                        while done < part_kt:
                            cnt = min(MAX_G, part_kt - done)
                            co = kt_base + part_off + done
                            nc.gpsimd.dma_gather(
                                g_t[:, part_off + done: part_off + done + cnt, :],
                                h_src[src_off: min(src_off + SPLIT, N), :],
                                idx_t[:, co * 8:(co + cnt) * 8],
                                cnt * 128, cnt * 128, H)
                            done += cnt
                    # --- one-hot build (one DVE op) ---
                    a_t = ahot.tile([128, KT, 128], f32, tag="at")
                    nc.vector.tensor_tensor(
                        out=a_t[:],
                        in0=iota_t[:, None, :].to_broadcast([128, KT, 128]),
                        in1=dstrel_t[:, kt_base:kt_base + KT, None].to_broadcast([128, KT, 128]),
                        op=mybir.AluOpType.is_equal)
                    # --- aggregation psum: agg[feat, dst] (self term folded into edges) ---
                    agg_p = ps.tile([128, 128], f32, tag="agg")
                    for kk in range(KT):
                        nc.tensor.matmul(out=agg_p[:], lhsT=g_t[:, kk, :], rhs=a_t[:, kk, :],
                                         start=(kk == 0), stop=(kk == KT - 1))
                    aggT = work.tile([128, 128], f32, tag="aggT")
                    nc.vector.tensor_copy(out=aggT[:], in_=agg_p[:])
                    # --- MLP ---
                    if final:
                        z_p = psm.tile([128, 128], f32, tag="z1")
                        nc.tensor.matmul(out=z_p[:OUT, :], lhsT=wl_t[:], rhs=aggT[:],
                                         start=True, stop=True)
                        z_t = work.tile([128, 128], f32, tag="zt")
                        nc.scalar.activation(out=z_t[:OUT, :], in_=z_p[:OUT, :],
                                             func=mybir.ActivationFunctionType.Sigmoid,
                                             bias=bl_t[:], scale=1.0)
                    else:
                        t1_p = psm.tile([128, 128], f32, tag="z1")
                        nc.tensor.matmul(out=t1_p[:], lhsT=wa_t[:, layer, :], rhs=aggT[:],
                                         start=True, stop=True)
                        t1 = work.tile([128, 128], f32, tag="t1")
                        nc.scalar.activation(out=t1[:], in_=t1_p[:],
                                             func=mybir.ActivationFunctionType.Relu,
                                             bias=ba_t[:, layer, None], scale=1.0)
                        z2_p = psm.tile([128, 128], f32, tag="z2")
                        nc.tensor.matmul(out=z2_p[:], lhsT=wb_t[:, layer, :], rhs=t1[:],
                                         start=True, stop=True)
                        z_t = work.tile([128, 128], f32, tag="zt")
                        nc.scalar.activation(out=z_t[:], in_=z2_p[:],
                                             func=mybir.ActivationFunctionType.Relu,
                                             bias=bb_t[:, layer, None], scale=1.0)
                    # --- transpose z -> node-major, write out ---
                    zT_p = psm.tile([128, 128], f32, tag="zT")
                    nc.tensor.matmul(out=zT_p[:, :wcols], lhsT=z_t[:wcols, :], rhs=ident_t[:wcols, :wcols],
                                     start=True, stop=True)
                    zz = work.tile([128, 128], f32, tag="zz")
                    nc.vector.tensor_copy(out=zz[:, :wcols], in_=zT_p[:, :wcols])
                    row0 = b * 128
                    if final:
                        nc.sync.dma_start(out=out_ext[row0:row0 + rows, :], in_=zz[:rows, :OUT])
                    else:
                        nc.sync.dma_start(out=ag_in[row0:row0 + rows, :], in_=zz[:rows, :H])
                sc.__exit__(None, None, None)
                if not final:
                    with nc.named_scope(f"AG{layer}"):
                        nc.gpsimd.collective_compute(
                            "AllGather", mybir.AluOpType.bypass,
                            replica_groups=[list(range(NCORES))],
                            ins=[ag_in[:, :]], outs=[h_dst[:, :]])
    nc.compile()
    return nc


def kernel(**inputs):
    from concourse.bass_utils import run_bass_kernel_spmd

    x = np.asarray(inputs["x"], np.float32)
    edge_index = np.asarray(inputs["edge_index"])
    idx_tables, dstrel_tables, K_LO, K_HI = _prep_edges(edge_index)

    key = (K_LO, K_HI)
    if key not in _CACHE:
        _CACHE[key] = _build(K_LO, K_HI)
    nc = _CACHE[key]

    wa = np.stack([inputs["w0a"]] + [inputs["wma"][i] for i in range(N_MID)]).astype(np.float32)
    wb = np.stack([inputs["w0b"]] + [inputs["wmb"][i] for i in range(N_MID)]).astype(np.float32)
    ba = np.stack([inputs["b0a"]] + [inputs["bma"][i] for i in range(N_MID)]).astype(np.float32)
    bb = np.stack([inputs["b0b"]] + [inputs["bmb"][i] for i in range(N_MID)]).astype(np.float32)

    iota = np.tile(np.arange(128, dtype=np.float32), (128, 1))
    ident = np.eye(128, dtype=np.float32)
    in_maps = []
    for k in range(NCORES):
        in_maps.append({
            "x": x, "idxs": idx_tables[k], "dstrel": dstrel_tables[k],
            "iota": iota, "ident": ident,
            "wa": wa, "wb": wb, "ba": ba, "bb": bb,
            "wl": inputs["wl"].astype(np.float32), "bl": inputs["bl"].astype(np.float32),
        })
    kernel._last = (nc, in_maps)  # test.py hook for traced re-runs
    res = run_bass_kernel_spmd(nc, in_maps, core_ids=list(range(NCORES)))
    out = np.concatenate([res.results[k]["out"] for k in range(NCORES)], axis=0)
    return out.astype(np.float32)

